# revision 14
# baseline (speedup 1.0000x reference)
"""AttentionBlock (GroupNorm + QKV 1x1 + 8-head attention + proj + residual)
as a Bass/Tile SPMD kernel for 8 Trainium2 NeuronCores.

Sharding: b*heads = 16 attention heads over 8 cores -> 2 heads/core.
GroupNorm + QKV input work is replicated within each 4-core batch group;
the output projection partial sums are combined with a 4-core ReduceScatter
(over channels), each core emitting a [128, 4096] slice of the output.

Math notes (all exact softmax-invariance rewrites of the reference):
 - k-bias dropped (adds a per-row constant to logits -> softmax invariant)
 - v-bias + proj-bias folded into a host-computed per-channel constant
   (cbias = w_proj @ b_v + b_proj) added after the reduce-scatter
 - attention scale folded into the q-side weights/bias on host
 - no max-subtraction in softmax: logits are O(+-10), exp is fp32-safe
"""

import math
import os

os.environ.setdefault("JAX_PLATFORMS", "")

import ml_dtypes
import numpy as np

import concourse.bass as bass
import concourse.mybir as mybir
import concourse.tile as tile
from concourse.bass_utils import run_bass_kernel_spmd
from concourse.vector_clock import ScopedClock

F32 = mybir.dt.float32
BF16 = mybir.dt.bfloat16
AX = mybir.AxisListType.X
ALU = mybir.AluOpType
AF = mybir.ActivationFunctionType

B, C, H, W = 2, 512, 64, 64
L = H * W                  # 4096
HEADS = 8
CH = C // HEADS            # 64
GROUPS = 32
GPT = GROUPS // 4          # groups per 128-channel tile = 8
CPG = C // GROUPS          # channels per group = 16
EPS = 1e-6
SCALE = 1.0 / math.sqrt(math.sqrt(CH))

P = 128
KT = C // P                # 4 contraction tiles
NT = L // 512              # 8 free-dim chunks of 512
NS = L // P                # 32 s-tiles
TB = 512                   # t-block size
GSZ = 3                    # (si, h)-chunks per exp group


class SplitDrainTileContext(tile.TileContext):
    """TileContext whose final drain is split into single-wait drains (this
    toolchain's walrus rejects CTRL instructions with >1 sync wait)."""

    def _drain_and_barrier(self, tick_clock, wait_clock):
        g = tick_clock.global_clock
        entries = [(p, g[p]) for p in range(len(g)) if g[p] > 0]
        for proc, tick in entries:
            partial = ScopedClock()
            partial.require_at_least(None, proc, tick)
            d = self.nc.sync.drain()
            wait_clock.add_sem_waits(d.ins, partial)
        if not entries:
            d = self.nc.sync.drain()
            wait_clock.add_sem_waits(d.ins, ScopedClock({None: g}))
        self.nc.all_engine_barrier()
        assert self.sems is not None
        popped = self.nc._tile_sem_poison_stack.pop()
        assert popped is self._sem_poison
        self.nc.clear_and_free_semaphores(list(self.sems.allocated().values()))
        self.nc.all_engine_barrier()


def _emit(nc, tc, ctx_pools, dbg=False):
    """Emit the per-core program. All per-core differences come from inputs."""
    if dbg:
        dbg_xn0 = nc.declare_dram_parameter("dbg_xn0", [P, L], BF16, isOutput=True)
        dbg_q = nc.declare_dram_parameter("dbg_q", [P, L], BF16, isOutput=True)
        dbg_k = nc.declare_dram_parameter("dbg_k", [P, L], BF16, isOutput=True)
        dbg_vt0 = nc.declare_dram_parameter("dbg_vt0", [P, 130], BF16, isOutput=True)
        dbg_e0 = nc.declare_dram_parameter("dbg_e0", [P, 1536], BF16, isOutput=True)
        dbg_a = nc.declare_dram_parameter("dbg_a", [P, L], BF16, isOutput=True)

    xb = nc.declare_dram_parameter("xb", [KT, P, L], F32, isOutput=False)
    wqkvT = nc.declare_dram_parameter("wqkvT", [KT, P, 384], BF16, isOutput=False)
    bq = nc.declare_dram_parameter("bq", [P, 1], F32, isOutput=False)
    wprojT = nc.declare_dram_parameter("wprojT", [P, C], BF16, isOutput=False)
    gamma_t = nc.declare_dram_parameter("gamma_t", [KT, P, 1], F32, isOutput=False)
    beta_t = nc.declare_dram_parameter("beta_t", [KT, P, 1], F32, isOutput=False)
    gmask = nc.declare_dram_parameter("gmask", [P, GPT], F32, isOutput=False)
    gmaskT = nc.declare_dram_parameter("gmaskT", [GPT, P], F32, isOutput=False)
    cbias = nc.declare_dram_parameter("cbias", [P, 1], F32, isOutput=False)
    xres = nc.declare_dram_parameter("xres", [P, L], F32, isOutput=False)
    ident = nc.declare_dram_parameter("ident", [P, P], BF16, isOutput=False)
    out = nc.declare_dram_parameter("out", [P, L], F32, isOutput=True)

    # ---------------- long-lived pools ----------------
    cpool = ctx_pools.enter_context(tc.tile_pool(name="consts", bufs=1))
    w_t = []
    for kt in range(KT):
        wt = cpool.tile([P, 384], BF16, name=f"w{kt}")
        nc.sync.dma_start(wt[:], wqkvT[kt])
        w_t.append(wt)
    wp_t = cpool.tile([P, C], BF16, name="wp")
    nc.sync.dma_start(wp_t[:], wprojT[:])
    bq_t = cpool.tile([P, 1], F32, name="bqt")
    nc.sync.dma_start(bq_t[:], bq[:])
    gm_t = cpool.tile([P, GPT], F32, name="gmt")
    nc.sync.dma_start(gm_t[:], gmask[:])
    gmT_t = cpool.tile([GPT, P], F32, name="gmTt")
    nc.sync.dma_start(gmT_t[:], gmaskT[:])
    cb_t = cpool.tile([P, 1], F32, name="cbt")
    nc.sync.dma_start(cb_t[:], cbias[:])
    ones_t = cpool.tile([1, 64], F32, name="onest")
    nc.gpsimd.memset(ones_t[:], 1.0)
    eps_t = cpool.tile([GPT, 1], F32, name="epst")
    nc.gpsimd.memset(eps_t[:], EPS)
    ident_t = cpool.tile([P, P], BF16, name="identt")
    nc.sync.dma_start(ident_t[:], ident[:])
    ga_t, be_t = [], []
    for kt in range(KT):
        g = cpool.tile([P, 1], F32, name=f"ga{kt}")
        nc.sync.dma_start(g[:], gamma_t[kt])
        ga_t.append(g)
        b = cpool.tile([P, 1], F32, name=f"be{kt}")
        nc.sync.dma_start(b[:], beta_t[kt])
        be_t.append(b)

    qkpool = ctx_pools.enter_context(tc.tile_pool(name="qk", bufs=1))
    q_both = qkpool.tile([P, L], BF16, name="q_both")
    k_both = qkpool.tile([P, L], BF16, name="k_both")
    a_both = qkpool.tile([P, L], BF16, name="a_both")

    vtpool = ctx_pools.enter_context(tc.tile_pool(name="vt", bufs=1))
    vt_t = [vtpool.tile([P, 130], BF16, name=f"vt{si}") for si in range(NS)]

    # ---------------- phase 1: load x, GroupNorm, QKV, vT ----------------
    with tc.tile_pool(name="ph1", bufs=1) as ph1, \
         tc.tile_pool(name="ph1ps", bufs=2, space="PSUM") as ph1ps:
        xn_t = []
        for kt in range(KT):
            x_t = ph1.tile([P, L], F32, name=f"x{kt}", tag="x", bufs=2)
            nc.sync.dma_start(x_t[:], xb[kt])
            # per-partition sum and sum-of-squares over L
            sums = ph1.tile([P, 2], F32, name=f"sums{kt}", tag="sums", bufs=2)
            scratch = ph1.tile([P, L], F32, name=f"scr{kt}", tag="scr", bufs=2)
            nc.scalar.activation(scratch[:], x_t[:], AF.Identity,
                                 accum_out=sums[:, 0:1])
            scratch2 = ph1.tile([P, L], F32, name=f"scr2_{kt}", tag="scr2",
                                bufs=2)
            nc.scalar.activation(scratch2[:], x_t[:], AF.Square,
                                 accum_out=sums[:, 1:2])
            # group stats: [8, 2] = mask^T @ sums
            gs_ps = ph1ps.tile([GPT, 2], F32, name=f"gs{kt}", tag="gs")
            nc.tensor.matmul(gs_ps[:], gm_t[:], sums[:], start=True, stop=True)
            gsm = ph1.tile([GPT, 2], F32, name=f"gsm{kt}", tag="gsm", bufs=2)
            nc.vector.tensor_scalar_mul(gsm[:], gs_ps[:], 1.0 / (CPG * L))
            var = ph1.tile([GPT, 1], F32, name=f"var{kt}", tag="var", bufs=2)
            nc.vector.tensor_tensor(var[:], gsm[:, 0:1], gsm[:, 0:1], op=ALU.mult)
            nc.vector.tensor_tensor(var[:], gsm[:, 1:2], var[:], op=ALU.subtract)
            sd = ph1.tile([GPT, 1], F32, name=f"sd{kt}", tag="sd", bufs=2)
            nc.scalar.activation(sd[:], var[:], AF.Sqrt, bias=eps_t[:])
            grp = ph1.tile([GPT, 2], F32, name=f"grp{kt}", tag="grp", bufs=2)
            nc.vector.reciprocal(grp[:, 0:1], sd[:])
            nc.vector.tensor_copy(grp[:, 1:2], gsm[:, 0:1])
            # expand group -> per-partition (rstd, mean)
            pp_ps = ph1ps.tile([P, 2], F32, name=f"pp{kt}", tag="pp")
            nc.tensor.matmul(pp_ps[:], gmT_t[:], grp[:], start=True, stop=True)
            A = ph1.tile([P, 1], F32, name=f"A{kt}", tag="A", bufs=2)
            nc.vector.tensor_tensor(A[:], pp_ps[:, 0:1], ga_t[kt][:], op=ALU.mult)
            Bt = ph1.tile([P, 1], F32, name=f"B{kt}", tag="B", bufs=2)
            nc.vector.tensor_tensor(Bt[:], pp_ps[:, 1:2], A[:], op=ALU.mult)
            nc.vector.tensor_tensor(Bt[:], be_t[kt][:], Bt[:], op=ALU.subtract)
            xn = ph1.tile([P, L], BF16, name=f"xn{kt}")
            nc.vector.tensor_scalar(xn[:], x_t[:], A[:], Bt[:],
                                    op0=ALU.mult, op1=ALU.add)
            if dbg and kt == 0:
                nc.sync.dma_start(dbg_xn0[:], xn[:])
            xn_t.append(xn)

        # QKV (q and k rows only; v goes straight to vT)
        with tc.tile_pool(name="qkvps", bufs=2, space="PSUM") as qkvps:
            v_both = qkpool.tile([P, L], BF16, name="v_both")
            for j, (dst, bias) in enumerate(((q_both, bq_t), (k_both, None),
                                             (v_both, None))):
                for t in range(NT):
                    ps = qkvps.tile([P, 512], F32, name=f"qkv{j}_{t}", tag="qkvps")
                    for kt in range(KT):
                        nc.tensor.matmul(
                            ps[:],
                            w_t[kt][:, j * P:(j + 1) * P],
                            xn_t[kt][:, t * 512:(t + 1) * 512],
                            start=(kt == 0), stop=(kt == KT - 1))
                    if bias is not None:
                        nc.vector.tensor_scalar_add(
                            dst[:, t * 512:(t + 1) * 512], ps[:], bias[:])
                    else:
                        nc.vector.tensor_copy(
                            dst[:, t * 512:(t + 1) * 512], ps[:])
            # vT: [s, c] tiles for both heads + ones columns (PE transpose)
            for si in range(NS):
                vps = qkvps.tile([P, P], BF16, name=f"vps{si}", tag="vps")
                nc.tensor.transpose(vps[:], v_both[:, si * P:(si + 1) * P],
                                    ident_t[:])
                vt = vt_t[si]
                nc.vector.memset(vt[:, 64:65], 1.0)
                nc.vector.memset(vt[:, 129:130], 1.0)
                nc.vector.tensor_copy(vt[:, 0:64], vps[:, 0:64])
                nc.vector.tensor_copy(vt[:, 65:129], vps[:, 64:128])
                if dbg and si == 0:
                    nc.sync.dma_start(dbg_vt0[:], vt[:])
            if dbg:
                nc.sync.dma_start(dbg_q[:], q_both[:])
                nc.sync.dma_start(dbg_k[:], k_both[:])

    # ------- phase 2+3: attention main loop with fused proj / RS / residual -
    chunks = [(c // 2, c % 2) for c in range(2 * NS)]  # (si, h), si-major
    groups = [chunks[i:i + GSZ] for i in range(0, len(chunks), GSZ)]
    dbg_e_done = [False]
    rgroups = [[0, 1, 2, 3], [4, 5, 6, 7]]
    with tc.tile_pool(name="epool", bufs=1) as epool, \
         tc.tile_pool(name="qkps", bufs=2, space="PSUM") as qkps, \
         tc.tile_pool(name="avps", bufs=1, space="PSUM") as avps, \
         tc.tile_pool(name="nrm", bufs=1) as nrm, \
         tc.tile_pool(name="stg", bufs=1) as stg, \
         tc.tile_pool(name="dram", bufs=1, space="DRAM") as dpool:
        hproj_t = [dpool.tile([C, TB], F32, name=f"hproj{tb}")
                   for tb in range(NT)]
        rs_t = [dpool.tile([P, TB], F32, name=f"rs{tb}") for tb in range(NT)]
        csd = {(tb, h): dpool.tile([1, TB], F32, name=f"csd{tb}_{h}")
               for tb in range(NT) for h in range(2)}
        for tb in range(NT):
            tsl = slice(tb * TB, (tb + 1) * TB)
            av = [avps.tile([65, 512], F32, name=f"av{h}_{tb}", tag=f"av{h}")
                  for h in range(2)]
            e_tiles = []
            for g in groups:
                qk = qkps.tile([P, GSZ * 512], F32, name=f"qk{tb}", tag="qk")
                for idx, (si, h) in enumerate(g):
                    nc.tensor.matmul(
                        qk[:, idx * 512:(idx + 1) * 512],
                        k_both[64 * h:64 * h + 64, si * P:(si + 1) * P],
                        q_both[64 * h:64 * h + 64, tsl],
                        start=True, stop=True)
                e_t = epool.tile([P, GSZ * 512], BF16, name=f"e{tb}", tag="e",
                                 bufs=24)
                n = len(g) * 512
                nc.scalar.activation(e_t[:, 0:n], qk[:, 0:n], AF.Exp)
                if dbg and not dbg_e_done[0]:
                    nc.sync.dma_start(dbg_e0[:], e_t[:])
                    dbg_e_done[0] = True
                e_tiles.append(e_t)
            # AV as two uninterrupted per-head accumulation chains
            for h in range(2):
                for si in range(NS):
                    c = 2 * si + h
                    e_t = e_tiles[c // GSZ]
                    idx = c % GSZ
                    nc.tensor.matmul(
                        av[h][:],
                        vt_t[si][:, 65 * h:65 * h + 65],
                        e_t[:, idx * 512:(idx + 1) * 512],
                        start=(si == 0), stop=(si == NS - 1))
            # softmax normalization, decoupled from the PSUM ring
            for h in range(2):
                au = nrm.tile([65, 512], F32, name=f"au{tb}_{h}",
                              tag=f"au{h}", bufs=2)
                nc.vector.tensor_copy(au[:], av[h][:])
                rcs = nrm.tile([1, 512], F32, name=f"rcs{tb}_{h}",
                               tag=f"rcs{h}", bufs=2)
                nc.vector.reciprocal(rcs[:], au[64:65, :])
                nc.sync.dma_start(csd[(tb, h)][:, :], rcs[:])
                rb = nrm.tile([64, 512], F32, name=f"rb{tb}_{h}",
                              tag=f"rb{h}", bufs=2)
                nc.sync.dma_start(rb[:],
                                  csd[(tb, h)][0:1, :].to_broadcast([64, TB]))
                nc.vector.tensor_tensor(
                    a_both[64 * h:64 * h + 64, tsl], au[0:64, :], rb[:],
                    op=ALU.mult)
            # output projection for this t-block (partial over our channels)
            for ot in range(KT):
                pj = avps.tile([P, 512], F32, name=f"pj{tb}_{ot}",
                               tag=f"av{ot % 2}")
                nc.tensor.matmul(pj[:], wp_t[:, ot * P:(ot + 1) * P],
                                 a_both[:, tsl], start=True, stop=True)
                st = stg.tile([P, 512], F32, name=f"st{tb}_{ot}", tag="st",
                              bufs=4)
                nc.vector.tensor_copy(st[:], pj[:])
                nc.sync.dma_start(hproj_t[tb][ot * P:(ot + 1) * P, :], st[:])
            # cross-core reduce of the partial projection (channel-sharded)
            nc.gpsimd.collective_compute(
                "ReduceScatter", ALU.add, replica_groups=rgroups,
                ins=[hproj_t[tb][:, :]], outs=[rs_t[tb][:, :]])
            rsb = stg.tile([P, 512], F32, name=f"rsb{tb}", tag="rsb", bufs=2)
            nc.sync.dma_start(rsb[:], rs_t[tb][:, :])
            xrb = stg.tile([P, 512], F32, name=f"xrb{tb}", tag="xrb", bufs=2)
            nc.sync.dma_start(xrb[:], xres[:, tsl])
            ob = stg.tile([P, 512], F32, name=f"ob{tb}", tag="ob", bufs=2)
            nc.vector.tensor_tensor(ob[:], rsb[:], xrb[:], op=ALU.add)
            nc.vector.tensor_scalar_add(ob[:], ob[:], cb_t[:])
            nc.sync.dma_start(out[:, tsl], ob[:])
    if dbg:
        nc.sync.dma_start(dbg_a[:], a_both[:])


def _split_waits(nc, limit=1):
    """This toolchain's walrus only encodes `limit` sync waits per
    instruction; hoist excess waits onto same-engine NOPs inserted just
    before the over-limit instruction (semantically a stricter stall)."""
    n_split = 0
    for f in nc.m.functions:
        for bb in f.blocks:
            live = bb.instructions
            new_list = []
            changed = False
            for inst in live:
                si = inst.sync_info
                if si is not None and len(si.on_wait) > limit:
                    waits = list(si.on_wait)
                    extra, keep = waits[:-limit], waits[-limit:]
                    for j in range(0, len(extra), limit):
                        nop = mybir.InstNoOp(
                            name=f"I-wsplit-{nc.next_id()}", ins=[], outs=[])
                        nop.engine = inst.engine
                        nop.sync_info = mybir.SyncInfo(
                            on_wait=extra[j:j + limit], on_update=[])
                        new_list.append(nop)
                        n_split += 1
                    inst.sync_info = mybir.SyncInfo(
                        on_wait=keep, on_update=list(si.on_update))
                    changed = True
                new_list.append(inst)
            if changed:
                live.clear()
                live.extend(new_list)
    return n_split


_CACHE = {}


def _build(dbg=False):
    key = ("nc", dbg)
    if key not in _CACHE:
        from contextlib import ExitStack
        nc = bass.Bass("TRN2", target_bir_lowering=False, debug=False,
                       num_devices=8)
        with SplitDrainTileContext(nc) as tc:
            with ExitStack() as pools:
                _emit(nc, tc, pools, dbg=dbg)
        _split_waits(nc)
        _CACHE[key] = nc
    return _CACHE[key]


def _host_inputs(x, gamma, beta, w_qkv, b_qkv, w_proj, b_proj):
    """Build the 8 per-core input maps."""
    xr = x.reshape(B, C, L)
    b_v = np.concatenate([b_qkv[192 * h + 128:192 * h + 192] for h in range(HEADS)])
    cbias_full = w_proj @ b_v + b_proj  # [C]
    gmask = np.zeros((P, GPT), np.float32)
    for p in range(P):
        gmask[p, p // CPG] = 1.0
    in_maps = []
    for core in range(8):
        b, r = divmod(core, 4)
        h0, h1 = 2 * r, 2 * r + 1
        qrows = np.concatenate([192 * h + np.arange(CH) for h in (h0, h1)])
        krows = qrows + CH
        vrows = krows + CH
        wsel = np.concatenate([w_qkv[qrows] * SCALE, w_qkv[krows] * SCALE,
                               w_qkv[vrows]], axis=0)  # [384, C]
        mych = np.concatenate([CH * h + np.arange(CH) for h in (h0, h1)])
        in_maps.append({
            "xb": np.ascontiguousarray(xr[b].reshape(KT, P, L)),
            "wqkvT": np.ascontiguousarray(
                wsel.T.reshape(KT, P, 384)).astype(ml_dtypes.bfloat16),
            "bq": np.ascontiguousarray(
                (b_qkv[qrows] * SCALE).reshape(P, 1)),
            "wprojT": np.ascontiguousarray(w_proj[:, mych].T).astype(ml_dtypes.bfloat16),
            "gamma_t": np.ascontiguousarray(gamma.reshape(KT, P, 1)),
            "beta_t": np.ascontiguousarray(beta.reshape(KT, P, 1)),
            "gmask": gmask,
            "gmaskT": np.ascontiguousarray(gmask.T),
            "cbias": np.ascontiguousarray(
                cbias_full[r * P:(r + 1) * P].reshape(P, 1)),
            "xres": np.ascontiguousarray(xr[b, r * P:(r + 1) * P]),
            "ident": np.eye(P, dtype=ml_dtypes.bfloat16),
        })
    return in_maps


def kernel(x, gamma, beta, w_qkv, b_qkv, w_proj, b_proj, _trace=False, _dbg=False):
    x = np.asarray(x, np.float32)
    gamma = np.asarray(gamma, np.float32)
    beta = np.asarray(beta, np.float32)
    w_qkv = np.asarray(w_qkv, np.float32)
    b_qkv = np.asarray(b_qkv, np.float32)
    w_proj = np.asarray(w_proj, np.float32)
    b_proj = np.asarray(b_proj, np.float32)

    nc = _build(dbg=_dbg)
    in_maps = _host_inputs(x, gamma, beta, w_qkv, b_qkv, w_proj, b_proj)
    res = run_bass_kernel_spmd(nc, in_maps, list(range(8)), trace=_trace)
    out = np.empty((B, C, L), np.float32)
    for core in range(8):
        b, r = divmod(core, 4)
        out[b, r * P:(r + 1) * P] = res.results[core]["out"]
    if _trace:
        kernel.last_results = res
    return out.reshape(B, C, H, W)
